# revision 6
# baseline (speedup 1.0000x reference)
"""BitLinear (int8-activation x ternary-weight) matmul on 8 TRN2 NeuronCores.

Full inputs: x [4, 4096, 2048] f32, weight [2048, 2048] f32.
Output: [4, 4096, 2048] fp16.

Strategy: data-parallel over the 16384 rows (2048 rows/core).
Weight prep is sharded: core i receives k-rows [i*256, (i+1)*256) of W^T,
computes its partial sum|W|, AllReduces the scalar to form sw = 1/mean|W|,
quantizes its slice to ternary bf16, and AllGathers the quantized qwT
(8 MiB bf16). Activations are quantized per-row to int8 values held in
bf16 (exact: |qx| <= 127; products accumulate in fp32 PSUM -> exact).
Host only reshapes/shards and transposes W (layout prep, no math).
"""

import numpy as np

import concourse.bass as bass
import concourse.mybir as mybir
import concourse.tile as tile
from concourse import bacc
from concourse.bass import ts
from concourse.bass_utils import run_bass_kernel_spmd
from concourse.masks import make_identity

N_CORES = 8
ROWS_TOTAL = 4 * 4096
K = 2048
N = 2048
WSLICE = K // N_CORES  # 256 k-rows of W^T per core
MAGIC = 12582912.0  # 1.5*2^23: fp32 round-to-nearest-even trick (both signs)

f32 = mybir.dt.float32
bf16 = mybir.dt.bfloat16
f16 = mybir.dt.float16
Alu = mybir.AluOpType
Act = mybir.ActivationFunctionType
AxX = mybir.AxisListType.X


def build(rows_per_core=ROWS_TOTAL // N_CORES):
    nc = bacc.Bacc(
        "TRN2", target_bir_lowering=False, debug=False, num_devices=N_CORES
    )
    x_ext = nc.declare_dram_parameter("x", [rows_per_core, K], f32, isOutput=False)
    wts_ext = nc.declare_dram_parameter("wts", [WSLICE, N], f32, isOutput=False)
    out_ext = nc.declare_dram_parameter(
        "out", [rows_per_core, N], f16, isOutput=True
    )
    # collective bounce buffers
    cc_in_sum = nc.dram_tensor("cc_in_sum", [1, 1], f32)
    cc_out_sum = nc.dram_tensor("cc_out_sum", [1, 1], f32, addr_space="Shared")
    cc_in_qw = nc.dram_tensor("cc_in_qw", [WSLICE, N], bf16)
    cc_out_qw = nc.dram_tensor("cc_out_qw", [K, N], bf16, addr_space="Shared")

    KT = K // 128
    MT = rows_per_core // 128
    NQ = N // 512
    WT = WSLICE // 128  # 2
    groups = [list(range(N_CORES))]

    with tile.TileContext(nc) as tc:
        with (
            tc.tile_pool(name="big", bufs=3) as big,  # [128,K] f32 x loads
            tc.tile_pool(name="wsl", bufs=WT) as wslp,  # W slice, kept in SBUF
            tc.tile_pool(name="scaled", bufs=2) as scaled,  # [128,K] f32 ACT out
            tc.tile_pool(name="qtmp", bufs=2) as qtmp,  # rounded f32 / qx bf16
            tc.tile_pool(name="qxt", bufs=2) as qxtp,  # [128,KT,128] bf16 x^T
            tc.tile_pool(name="outp", bufs=3) as outp,  # [128,N] f16 results
            tc.tile_pool(name="singles", bufs=1) as singles,
            tc.tile_pool(name="small", bufs=6) as small,  # [128,1] stats
            tc.tile_pool(name="pacc", bufs=6, space="PSUM") as pacc,
            tc.tile_pool(name="pt", bufs=2, space="PSUM") as pt,
        ):
            ident = singles.tile([128, 128], bf16)
            make_identity(nc, ident)
            ones_col = singles.tile([128, 1], f32)
            nc.vector.memset(ones_col, 1.0)
            ones_row = singles.tile([1, 128], f32)
            nc.vector.memset(ones_row, 1.0)
            qwT = singles.tile([128, KT, N], bf16)
            wsums = singles.tile([128, WT], f32)

            # ---- W slice: load + partial |W| sum
            wsl_tiles = []
            for wt_i in range(WT):
                wsl_t = wslp.tile([128, N], f32, tag="wsl", name=f"wsl{wt_i}")
                nc.sync.dma_start(out=wsl_t, in_=wts_ext[ts(wt_i, 128), :])
                wsl_tiles.append(wsl_t)
                nc.vector.tensor_reduce(
                    out=wsums[:, wt_i : wt_i + 1],
                    in_=wsl_t,
                    axis=AxX,
                    op=Alu.add,
                    apply_absolute_value=True,
                )
            wtot = small.tile([128, 1], f32, tag="small")
            nc.vector.tensor_reduce(out=wtot, in_=wsums, axis=AxX, op=Alu.add)
            ptot = pt.tile([1, 1], f32, tag="pt")
            nc.tensor.matmul(ptot, lhsT=ones_col, rhs=wtot, start=True, stop=True)
            s_part = small.tile([1, 1], f32, tag="s1")
            nc.vector.tensor_copy(out=s_part, in_=ptot)
            nc.sync.dma_start(out=cc_in_sum[:, :], in_=s_part)
            nc.gpsimd.collective_compute(
                "AllReduce",
                Alu.add,
                replica_groups=groups,
                ins=[cc_in_sum[:, :]],
                outs=[cc_out_sum[:, :]],
            )
            s_total = small.tile([1, 1], f32, tag="s1")
            nc.sync.dma_start(out=s_total, in_=cc_out_sum[:, :])
            # meanc = max(mean|W|, 1e-5); sw = 1/meanc; q = meanc/127
            s_meanc = small.tile([1, 1], f32, tag="s1")
            nc.vector.tensor_scalar(
                out=s_meanc,
                in0=s_total,
                scalar1=1.0 / (K * N),
                scalar2=1e-5,
                op0=Alu.mult,
                op1=Alu.max,
            )
            s_sw = small.tile([1, 1], f32, tag="s1")
            nc.vector.reciprocal(out=s_sw, in_=s_meanc)
            s_q = small.tile([1, 1], f32, tag="s1")
            nc.vector.tensor_scalar_mul(out=s_q, in0=s_meanc, scalar1=1.0 / 127.0)
            # broadcast scalars to all 128 partitions via PE outer product
            pb = pt.tile([128, 1], f32, tag="pt")
            nc.tensor.matmul(pb, lhsT=ones_row, rhs=s_sw, start=True, stop=True)
            sw_b = singles.tile([128, 1], f32)
            nc.vector.tensor_copy(out=sw_b, in_=pb)
            pb2 = pt.tile([128, 1], f32, tag="pt")
            nc.tensor.matmul(pb2, lhsT=ones_row, rhs=s_q, start=True, stop=True)
            q_b = singles.tile([128, 1], f32)
            nc.vector.tensor_copy(out=q_b, in_=pb2)

            # ---- quantize own W slice -> bf16, send, gather full qwT
            for wt_i in range(WT):
                ws = scaled.tile([128, N], f32, tag="scaled")
                nc.scalar.activation(
                    out=ws, in_=wsl_tiles[wt_i], func=Act.Copy, scale=sw_b
                )
                wr = qtmp.tile([128, N], f32, tag="qtmp")
                nc.vector.tensor_scalar(
                    out=wr, in0=ws, scalar1=MAGIC, scalar2=-MAGIC,
                    op0=Alu.add, op1=Alu.add,
                )
                wq = qtmp.tile([128, N], bf16, tag="qtmp")
                nc.vector.tensor_scalar(
                    out=wq, in0=wr, scalar1=-1.0, scalar2=1.0,
                    op0=Alu.max, op1=Alu.min,
                )
                nc.sync.dma_start(out=cc_in_qw[ts(wt_i, 128), :], in_=wq)
            nc.gpsimd.collective_compute(
                "AllGather",
                Alu.bypass,
                replica_groups=groups,
                ins=[cc_in_qw[:, :]],
                outs=[cc_out_qw[:, :]],
            )
            for kt in range(KT):
                nc.sync.dma_start(
                    out=qwT[:, kt, :], in_=cc_out_qw[ts(kt, 128), :]
                )

            # ---- main loop over row tiles
            for mi in range(MT):
                x_t = big.tile([128, K], f32, tag="big")
                nc.sync.dma_start(out=x_t, in_=x_ext[ts(mi, 128), :])
                amax = small.tile([128, 1], f32, tag="small")
                nc.vector.tensor_reduce(
                    out=amax, in_=x_t, axis=AxX, op=Alu.max,
                    apply_absolute_value=True,
                )
                amc = small.tile([128, 1], f32, tag="small")
                nc.vector.tensor_scalar_max(out=amc, in0=amax, scalar1=1e-5)
                rec = small.tile([128, 1], f32, tag="small")
                nc.vector.reciprocal(out=rec, in_=amc)
                si = small.tile([128, 1], f32, tag="small")
                nc.vector.tensor_scalar_mul(out=si, in0=rec, scalar1=127.0)
                cs = small.tile([128, 1], f32, tag="small")
                nc.vector.tensor_mul(cs, amc, q_b)  # (amax/127)*meanc

                xs = scaled.tile([128, K], f32, tag="scaled")
                nc.scalar.activation(out=xs, in_=x_t, func=Act.Copy, scale=si)
                qx = qtmp.tile([128, K], bf16, tag="qtmp")
                nc.vector.tensor_scalar(
                    out=qx, in0=xs, scalar1=MAGIC, scalar2=-MAGIC,
                    op0=Alu.add, op1=Alu.add,
                )
                qxT = qxtp.tile([128, KT, 128], bf16, tag="qxt")
                for kt in range(KT):
                    ptr = pt.tile([128, 128], bf16, tag="pt")
                    nc.tensor.transpose(ptr, qx[:, ts(kt, 128)], ident)
                    nc.vector.tensor_copy(out=qxT[:, kt, :], in_=ptr)

                accs = [
                    pacc.tile([128, 512], f32, tag="acc", name=f"acc_{mi}_{i}")
                    for i in range(NQ)
                ]
                for nq in range(NQ):
                    for kt in range(KT):
                        nc.tensor.matmul(
                            accs[nq],
                            lhsT=qxT[:, kt, :],
                            rhs=qwT[:, kt, ts(nq, 512)],
                            start=(kt == 0),
                            stop=(kt == KT - 1),
                        )
                o_t = outp.tile([128, N], f16, tag="outp")
                for nq in range(NQ):
                    nc.scalar.activation(
                        out=o_t[:, ts(nq, 512)], in_=accs[nq],
                        func=Act.Copy, scale=cs,
                    )
                nc.sync.dma_start(out=out_ext[ts(mi, 128), :], in_=o_t)

    nc.compile()
    return nc


_NC_CACHE = {}


def _get_nc(rows_per_core):
    if rows_per_core not in _NC_CACHE:
        _NC_CACHE[rows_per_core] = build(rows_per_core)
    return _NC_CACHE[rows_per_core]


def run(x, weight, **spmd_kwargs):
    x = np.ascontiguousarray(np.asarray(x, dtype=np.float32))
    weight = np.asarray(weight, dtype=np.float32)
    b, s, k = x.shape
    rows = b * s
    rpc = rows // N_CORES
    xr = x.reshape(rows, k)
    wt = np.ascontiguousarray(weight.T)
    nc = _get_nc(rpc)
    in_maps = [
        {
            "x": xr[i * rpc : (i + 1) * rpc],
            "wts": np.ascontiguousarray(wt[i * WSLICE : (i + 1) * WSLICE]),
        }
        for i in range(N_CORES)
    ]
    res = run_bass_kernel_spmd(
        nc, in_maps, core_ids=list(range(N_CORES)), **spmd_kwargs
    )
    out = np.concatenate(
        [res.results[i]["out"] for i in range(N_CORES)], axis=0
    )
    return out.reshape(b, s, N), res


def kernel(x, weight):
    out, _ = run(x, weight)
    return out
